# revision 7
# baseline (speedup 1.0000x reference)
"""Trainium2 Bass kernel for nn_Attention_84516366450883 (gnn message passing).

Computation (reference):
    leave_emb = W_emb[leaves]          # [N, A, E]
    anc_emb   = W_emb[ancestors]       # [N, A, E]
    mlp  = tanh(concat(leave_emb, anc_emb) @ W_attention + b)   # [N, A, ATT]
    pre  = mlp @ v                     # [N, A]
    attn = softmax(pre, axis=1)
    out  = einsum('nae,na->ne', anc_emb, attn)                  # [N, E]

Sharding: data-parallel over N across 8 cores. W_emb + attention params
replicated; each core gathers its shard's leaf/ancestor embedding rows via
indirect DMA and computes locally. No collectives.

v2 design (vs the 16-gathers/tile baseline at 2.28 ms):
  - ONE batched indirect DMA per chunk of G=7 tiles: the offset AP is
    [128, G*16] so a single SWDGE instruction gathers 14336 rows,
    amortizing the ~1 us fixed descriptor-gen cost that dominated v1.
  - W_emb, W_attention, v are fed as bf16 (host-side cast): halves the
    gather traffic and runs the PE path at 1 cycle/row instead of
    fp32 LOW_HIGH pairs.
  - Transposes write a shared bf16 PSUM tile 4 slots at a time; the
    PSUM->SBUF copies are batched [128,512] and spread across
    DVE/ACT/Pool to balance engines.
  - z computed as 4 big matmuls (rhs free dim 512) instead of 16.
  - Output staged per chunk and stored with one DMA per chunk.
"""

import sys

if "/opt/trn_rl_repo" not in sys.path:
    sys.path.insert(0, "/opt/trn_rl_repo")

import numpy as np

VOCAB, EMB, ATT = 100000, 128, 128
N_CODES, N_ANC = 100000, 8
NCORES = 8
NSH = N_CODES // NCORES            # 12500 codes per core
TILES = (NSH + 127) // 128         # 98
NPAD = TILES * 128                 # 12544
NSLOT = 2 * N_ANC                  # 16 gathered rows per code
G = 7                              # tiles per gather chunk
NCH = TILES // G                   # 14 chunks

_nc_cache = {}


def _build(num_devices=NCORES):
    import concourse.bacc as bacc
    import concourse.tile as tile
    from concourse import bass, mybir
    from concourse.masks import make_identity

    f32 = mybir.dt.float32
    bf16 = mybir.dt.bfloat16
    i32 = mybir.dt.int32
    Act = mybir.ActivationFunctionType

    nc = bacc.Bacc("TRN2", target_bir_lowering=False, debug=False,
                   num_devices=num_devices)
    w_emb = nc.dram_tensor("w_emb", (VOCAB, EMB), bf16, kind="ExternalInput").ap()
    w_att = nc.dram_tensor("w_att", (2 * EMB, ATT), bf16, kind="ExternalInput").ap()
    b_att = nc.dram_tensor("b_att", (1, ATT), f32, kind="ExternalInput").ap()
    v_att = nc.dram_tensor("v_att", (1, ATT), bf16, kind="ExternalInput").ap()
    idx = nc.dram_tensor("idx", (NPAD, NSLOT), i32, kind="ExternalInput").ap()
    out = nc.dram_tensor("out", (NPAD, EMB), f32, kind="ExternalOutput").ap()

    with tile.TileContext(nc) as tc:
        with (
            tc.tile_pool(name="const", bufs=1) as cpool,
            tc.tile_pool(name="gat", bufs=3) as gpool,
            tc.tile_pool(name="tr", bufs=2) as tpool,
            tc.tile_pool(name="mlp", bufs=2) as mpool,
            tc.tile_pool(name="sm", bufs=3) as smpool,
            tc.tile_pool(name="ws", bufs=2) as wpool,
            tc.tile_pool(name="st", bufs=2) as stpool,
            tc.tile_pool(name="pst", bufs=2, space="PSUM") as pst_pool,
            tc.tile_pool(name="psz", bufs=4, space="PSUM") as psz_pool,
            tc.tile_pool(name="psp", bufs=2, space="PSUM") as psp_pool,
        ):
            # idx preload first: the first gather can only start once its
            # offsets are in SBUF.
            idx_sb = cpool.tile([128, TILES * NSLOT], i32)
            nc.sync.dma_start(
                idx_sb[:].rearrange("p (t s) -> p t s", s=NSLOT),
                idx.rearrange("(t p) s -> p t s", p=128))
            ident = cpool.tile([128, 128], bf16)
            make_identity(nc, ident[:])
            wl = cpool.tile([EMB, ATT], bf16)
            nc.sync.dma_start(wl[:], w_att[0:EMB, :])
            wa = cpool.tile([EMB, ATT], bf16)
            nc.sync.dma_start(wa[:], w_att[EMB:2 * EMB, :])
            bias = cpool.tile([ATT, 1], f32)
            nc.sync.dma_start(bias[:], b_att.rearrange("a b -> b a"))
            vv = cpool.tile([ATT, 1], bf16)
            nc.sync.dma_start(vv[:], v_att.rearrange("a b -> b a"))

            for c in range(NCH):
                # --- gather the chunk: one instruction per tile (16 offset
                # columns = 2048 rows). Wider offset APs (64/112 cols)
                # sporadically corrupted rows on HW.
                g = gpool.tile([128, G * NSLOT * EMB], bf16, tag="g")
                cols = G * NSLOT            # 112 offsets per partition
                GW = 1                      # gather offset-AP width (bisect)
                for lo in range(0, cols, GW):
                    hi = lo + GW
                    nc.gpsimd.indirect_dma_start(
                        out=g[:, lo * EMB:hi * EMB],
                        out_offset=None,
                        in_=w_emb,
                        in_offset=bass.IndirectOffsetOnAxis(
                            ap=idx_sb[:, c * cols + lo:c * cols + hi], axis=0),
                    )

                ostage = stpool.tile([128, G * EMB], f32, tag="ost")
                for tl in range(G):
                    base = tl * NSLOT * EMB

                    # --- transpose 16 slots, 4 per PSUM tile --------------
                    gt = tpool.tile([128, NSLOT * EMB], bf16, tag="gt")
                    for grp in range(4):
                        ps = pst_pool.tile([128, 512], bf16, tag="pst")
                        for k in range(4):
                            s = grp * 4 + k
                            nc.tensor.transpose(
                                ps[:, k * 128:(k + 1) * 128],
                                g[:, base + s * EMB:base + (s + 1) * EMB],
                                ident[:])
                        dst = gt[:, grp * 512:(grp + 1) * 512]
                        if grp % 2 == 0:
                            nc.vector.tensor_copy(dst, ps[:])
                        else:
                            nc.scalar.copy(dst, ps[:])

                    # --- z = W_l.T @ LT + W_a.T @ AT (4 big matmuls) ------
                    z0 = psz_pool.tile([128, 512], f32, tag="z")
                    z1 = psz_pool.tile([128, 512], f32, tag="z")
                    nc.tensor.matmul(z0[:], lhsT=wl[:], rhs=gt[:, 0:512],
                                     start=True, stop=False)
                    nc.tensor.matmul(z0[:], lhsT=wa[:], rhs=gt[:, 1024:1536],
                                     start=False, stop=True)
                    nc.tensor.matmul(z1[:], lhsT=wl[:], rhs=gt[:, 512:1024],
                                     start=True, stop=False)
                    nc.tensor.matmul(z1[:], lhsT=wa[:], rhs=gt[:, 1536:2048],
                                     start=False, stop=True)

                    # --- mlp = tanh(z + b) --------------------------------
                    mlp = mpool.tile([128, N_ANC * ATT], bf16, tag="mlp")
                    nc.scalar.activation(mlp[:, 0:512], z0[:], Act.Tanh,
                                         bias=bias[:])
                    nc.scalar.activation(mlp[:, 512:1024], z1[:], Act.Tanh,
                                         bias=bias[:])

                    # --- pre[codes, j] = mlp_j.T @ v ----------------------
                    pre = psp_pool.tile([128, N_ANC], f32, tag="pre")
                    for j in range(N_ANC):
                        nc.tensor.matmul(pre[:, j:j + 1],
                                         lhsT=mlp[:, j * ATT:(j + 1) * ATT],
                                         rhs=vv[:], start=True, stop=True)

                    # --- softmax: weight by raw exp, scale by 1/sum after
                    # the ancestor reduction (saves the attn normalize).
                    ex = smpool.tile([128, N_ANC], bf16, tag="ex")
                    nc.scalar.activation(ex[:], pre[:], Act.Exp)
                    ssum = smpool.tile([128, 1], f32, tag="ssum")
                    nc.vector.reduce_sum(ssum[:], ex[:], axis=mybir.AxisListType.X)
                    rec = smpool.tile([128, 1], f32, tag="rec")
                    nc.vector.reciprocal(rec[:], ssum[:])

                    # --- weighted sum over ancestors ----------------------
                    anc = g[:, base + N_ANC * EMB:base + NSLOT * EMB]
                    ws = wpool.tile([128, N_ANC * EMB], bf16, tag="ws")
                    nc.gpsimd.tensor_mul(
                        ws[:].rearrange("p (a e) -> p a e", a=N_ANC),
                        anc.rearrange("p (a e) -> p a e", a=N_ANC),
                        ex[:].to_broadcast([128, N_ANC, EMB]),
                    )
                    ost = ostage[:, tl * EMB:(tl + 1) * EMB]
                    nc.vector.tensor_reduce(
                        ost,
                        ws[:].rearrange("p (a e) -> p e a", a=N_ANC),
                        axis=mybir.AxisListType.X, op=mybir.AluOpType.add)
                    nc.vector.tensor_mul(ost, ost,
                                         rec[:].to_broadcast([128, EMB]))

                # --- store the chunk's G output tiles ---------------------
                nc.sync.dma_start(
                    out[c * G * 128:(c + 1) * G * 128, :].rearrange(
                        "(t p) e -> p t e", p=128),
                    ostage[:].rearrange("p (t e) -> p t e", e=EMB))

    nc.compile()
    return nc


def _get_nc(num_devices=NCORES):
    key = num_devices
    if key not in _nc_cache:
        _nc_cache[key] = _build(num_devices)
    return _nc_cache[key]


def _to_bf16(a):
    import ml_dtypes
    return np.ascontiguousarray(
        np.asarray(a, dtype=np.float32).astype(ml_dtypes.bfloat16))


def _prep_in_maps(inputs):
    W_emb = _to_bf16(inputs["W_emb"])
    W_attention = _to_bf16(inputs["W_attention"])
    b_attention = np.ascontiguousarray(
        np.asarray(inputs["b_attention"], dtype=np.float32).reshape(1, ATT))
    v_attention = _to_bf16(
        np.asarray(inputs["v_attention"], dtype=np.float32).reshape(1, ATT))
    leaves = np.asarray(inputs["leaves"]).astype(np.int32)
    ancestors = np.asarray(inputs["ancestors"]).astype(np.int32)

    idx_all = np.concatenate([leaves, ancestors], axis=1)   # [N, 16]
    in_maps = []
    for c in range(NCORES):
        shard = idx_all[c * NSH:(c + 1) * NSH]
        pad = np.zeros((NPAD, NSLOT), dtype=np.int32)
        pad[:NSH] = shard
        in_maps.append({
            "w_emb": W_emb,
            "w_att": W_attention,
            "b_att": b_attention,
            "v_att": v_attention,
            "idx": np.ascontiguousarray(pad),
        })
    return in_maps


def run(inputs, trace=False, **kwargs):
    """Run on the 8 NeuronCores; returns (output [N, E] f32, BassKernelResults)."""
    from concourse import bass_utils
    nc = _get_nc()
    in_maps = _prep_in_maps(inputs)
    res = bass_utils.run_bass_kernel_spmd(
        nc, in_maps, core_ids=list(range(NCORES)), trace=trace, **kwargs)
    outs = [res.results[c]["out"][:NSH] for c in range(NCORES)]
    full = np.concatenate(outs, axis=0).astype(np.float32)
    return full, res


def kernel(**inputs) -> np.ndarray:
    full, _ = run(inputs, trace=False)
    return full
